# revision 17
# baseline (speedup 1.0000x reference)
"""OAdder2d_Q (oconv, 16-bit dorefa quant) as an 8-core Trainium2 Bass kernel.

Math: with ideal disks the op is a 3x3/pad1 conv with effective kernel
w_q * sin(phases)*(d0+d1)/2.  The tiny weight transform (tanh/dorefa +
phase fold) runs on host; the conv runs on device as 9 shifted matmuls
(one per kernel tap) accumulating in PSUM, operands in fp16.

The 16-bit input quantize round(clip(x)*65535)/65535 perturbs x by at most
7.6e-6 relative -- far below fp16's 2.4e-4 ulp -- so casting x straight to
fp16 is numerically indistinguishable from quantize-then-cast (verified:
6.7e-4 vs 6.4e-4 scale-relative error).  The input path is therefore a
single dtype-casting DMA into a zero-padded fp16 SBUF tile.

Sharding: data-parallel over batch, 32 images -> 4 per core, weights
replicated.
"""

import sys

if "/opt/trn_rl_repo" not in sys.path:
    sys.path.insert(0, "/opt/trn_rl_repo")

import numpy as np

import concourse.bacc as bacc
import concourse.mybir as mybir
from concourse.tile import TileContext
from concourse.bass_utils import run_bass_kernel_spmd

N_CORES = 8
B, C, O, K, H, W = 32, 128, 256, 3, 56, 56
PB = B // N_CORES              # images per core
HP, WP = H + 2, W + 2          # padded spatial
RB = 8                         # output rows per psum tile
NRB = H // RB                  # row blocks per image
QN = 65535.0                   # 2^16 - 1

f32 = mybir.dt.float32
f16 = mybir.dt.float16

_CACHE = {}


def _build_nc():
    nc = bacc.Bacc("TRN2", target_bir_lowering=False, debug=False,
                   num_devices=N_CORES)
    x = nc.dram_tensor("x", (PB, C, H, W), f32, kind="ExternalInput")
    w = nc.dram_tensor("w", (C, 9 * O), f16, kind="ExternalInput")
    y = nc.dram_tensor("y", (PB, O, H, W), f32, kind="ExternalOutput")

    # img0 input row chunks, halo-aligned so chunk k unlocks row-block k:
    # rb k's matmuls read padded rows [8k, 8k+10] = x rows [8k-1, 8k+9]
    CH0 = [(0, 10)] + [(8 * k + 2, 8 * k + 10) for k in range(1, NRB - 1)] \
        + [(8 * (NRB - 1) + 2, H)]
    with TileContext(nc) as tc:
        with tc.tile_pool(name="wp", bufs=1) as wp, \
             tc.tile_pool(name="xpp", bufs=2) as xpp, \
             tc.tile_pool(name="pp", bufs=6, space="PSUM") as pp, \
             tc.tile_pool(name="wup", bufs=1, space="PSUM") as wup, \
             tc.tile_pool(name="op", bufs=4) as outp:
            # PE warm-up: dummy matmuls with no data deps so the HAM clock
            # gate is at 8/8 by the time real matmuls start (and stays there
            # until the first image's data lands).
            wu_in = wp.tile([C, 64], f16)
            nc.vector.memset(wu_in, 0.0)
            wu_ps = wup.tile([32, 64], f32)
            for _ in range(76):
                nc.tensor.matmul(wu_ps, wu_in[:, :32], wu_in[:, :64],
                                 start=True, stop=True)
            # img0 chunk0 via fast HWDGE path (sync can't cast: stage fp32,
            # cast on DVE); remaining chunks via gpsimd casting DMA
            wt = wp.tile([C, 9 * O], f16)
            xs0 = wp.tile([C, 10, W], f32)
            nc.sync.dma_start(out=xs0, in_=x[0, :, 0:10, :])
            nc.scalar.dma_start(out=wt, in_=w[:, :])
            for img in range(PB):
                xp = xpp.tile([C, HP, WP], f16)
                nc.vector.memset(xp[:, 0, :], 0.0)
                nc.vector.memset(xp[:, HP - 1, :], 0.0)
                nc.vector.memset(xp[:, 1:HP - 1, 0], 0.0)
                nc.vector.memset(xp[:, 1:HP - 1, WP - 1], 0.0)
                # fp32 -> fp16 casting DMA straight into the padded tile
                if img == 0:
                    for ci, (r0, r1) in enumerate(CH0):
                        if ci == 0:
                            nc.vector.tensor_copy(
                                out=xp[:, 1:11, 1:W + 1], in_=xs0)
                            continue
                        nc.gpsimd.dma_start(
                            out=xp[:, r0 + 1:r1 + 1, 1:W + 1],
                            in_=x[img, :, r0:r1, :])
                else:
                    nc.gpsimd.dma_start(out=xp[:, 1:H + 1, 1:W + 1],
                                        in_=x[img, :, :, :])
                for rb in range(NRB):
                    for oh in range(O // 128):
                        ps = pp.tile([128, RB, W], f32)
                        for ki in range(K):
                            for kj in range(K):
                                kidx = ki * K + kj
                                rhs = xp[:, rb * RB + ki: rb * RB + ki + RB,
                                         kj: kj + W]
                                lhsT = wt[:, kidx * O + oh * 128:
                                          kidx * O + oh * 128 + 128]
                                nc.tensor.matmul(ps, lhsT, rhs,
                                                 start=(kidx == 0),
                                                 stop=(kidx == K * K - 1))
                        yt = outp.tile([128, RB, W], f32)
                        if img == PB - 1 and rb == NRB - 1:
                            # tail latency: drain the last PSUM tiles with
                            # both engines in parallel
                            nc.vector.tensor_copy(out=yt[:, :RB // 2, :],
                                                  in_=ps[:, :RB // 2, :])
                            nc.scalar.copy(out=yt[:, RB // 2:, :],
                                           in_=ps[:, RB // 2:, :])
                        elif oh % 2 == 0:
                            nc.vector.tensor_copy(out=yt, in_=ps)
                        else:
                            nc.scalar.copy(out=yt, in_=ps)
                        nc.sync.dma_start(
                            out=y[img, oh * 128:(oh + 1) * 128,
                                  rb * RB:(rb + 1) * RB, :],
                            in_=yt)
    nc.compile()
    return nc


def _prep_weights(weight, phases, disks):
    """dorefa weight quantize + fold phases/disks into the conv kernel."""
    t = np.tanh(weight.astype(np.float32))
    t = t / (2.0 * np.max(np.abs(t))) + 0.5
    wq = (np.round(t * QN) / np.float32(QN)).astype(np.float32)
    s = np.sin(phases.astype(np.float32))[0, 0]        # (C,K,K)
    d0 = disks[0, 0, ..., 0].astype(np.float32)
    d1 = disks[0, 0, ..., 1].astype(np.float32)
    k_mul = wq * (s * (d0 + d1) * 0.5)[None]           # (O,C,K,K)
    # lhsT layout: [c, kidx*O + o]
    wsb = np.ascontiguousarray(
        k_mul.transpose(1, 2, 3, 0).reshape(C, 9 * O)).astype(np.float16)
    coef = (d0 - d1) * 0.25                            # (C,K,K)
    return wsb, wq, coef


def _square_terms(x, wq, coef):
    """Generic-disk correction (zero for ideal disks): conv(x_q^2, coef)
    broadcast over O, plus per-O constant sum(w_q^2 * coef)."""
    xq = np.round(np.clip(x, 0.0, 1.0) * QN) / np.float32(QN)
    x2 = (xq * xq).astype(np.float32)
    bsz = x.shape[0]
    x2p = np.zeros((bsz, C, H + 2, W + 2), np.float32)
    x2p[:, :, 1:H + 1, 1:W + 1] = x2
    y_sq = np.zeros((bsz, H, W), np.float32)
    for ki in range(K):
        for kj in range(K):
            y_sq += np.einsum("bchw,c->bhw",
                              x2p[:, :, ki:ki + H, kj:kj + W],
                              coef[:, ki, kj], optimize=True)
    w_term = np.einsum("ockk,ckk->o", wq * wq, coef)
    return y_sq[:, None] + w_term[None, :, None, None]


def kernel(x, weight, phases, disks):
    x = np.asarray(x)
    wsb, wq, coef = _prep_weights(np.asarray(weight), np.asarray(phases),
                                  np.asarray(disks))
    if "nc" not in _CACHE:
        _CACHE["nc"] = _build_nc()
    nc = _CACHE["nc"]
    in_maps = [{"x": np.ascontiguousarray(x[c * PB:(c + 1) * PB]), "w": wsb}
               for c in range(N_CORES)]
    res = run_bass_kernel_spmd(nc, in_maps, list(range(N_CORES)))
    y = np.concatenate([res.results[c]["y"] for c in range(N_CORES)], axis=0)
    if np.any(coef != 0.0):
        y = y + _square_terms(x, wq, coef)
    return y.astype(np.float32)


# revision 18
# speedup vs baseline: 1.0155x; 1.0155x over previous
"""OAdder2d_Q (oconv, 16-bit dorefa quant) as an 8-core Trainium2 Bass kernel.

Math: with ideal disks the op is a 3x3/pad1 conv with effective kernel
w_q * sin(phases)*(d0+d1)/2.  The tiny weight transform (tanh/dorefa +
phase fold) runs on host; the conv runs on device as 9 shifted matmuls
(one per kernel tap) accumulating in PSUM, operands in fp16.

The 16-bit input quantize round(clip(x)*65535)/65535 perturbs x by at most
7.6e-6 relative -- far below fp16's 2.4e-4 ulp -- so casting x straight to
fp16 is numerically indistinguishable from quantize-then-cast (verified:
6.7e-4 vs 6.4e-4 scale-relative error).  The input path is therefore a
single dtype-casting DMA into a zero-padded fp16 SBUF tile.

Sharding: data-parallel over batch, 32 images -> 4 per core, weights
replicated.
"""

import sys

if "/opt/trn_rl_repo" not in sys.path:
    sys.path.insert(0, "/opt/trn_rl_repo")

import numpy as np

import concourse.bacc as bacc
import concourse.mybir as mybir
from concourse.tile import TileContext
from concourse.bass_utils import run_bass_kernel_spmd

N_CORES = 8
B, C, O, K, H, W = 32, 128, 256, 3, 56, 56
PB = B // N_CORES              # images per core
HP, WP = H + 2, W + 2          # padded spatial
RB = 8                         # output rows per psum tile
NRB = H // RB                  # row blocks per image
QN = 65535.0                   # 2^16 - 1

f32 = mybir.dt.float32
f16 = mybir.dt.float16

_CACHE = {}


def _build_nc():
    nc = bacc.Bacc("TRN2", target_bir_lowering=False, debug=False,
                   num_devices=N_CORES)
    x = nc.dram_tensor("x", (PB, C, H, W), f32, kind="ExternalInput")
    w = nc.dram_tensor("w", (C, 9 * O), f16, kind="ExternalInput")
    y = nc.dram_tensor("y", (PB, O, H, W), f32, kind="ExternalOutput")

    # img0 input row chunks, halo-aligned so chunk k unlocks row-block k:
    # rb k's matmuls read padded rows [8k, 8k+10] = x rows [8k-1, 8k+9]
    CH0 = [(0, 10)] + [(8 * k + 2, 8 * k + 10) for k in range(1, NRB - 1)] \
        + [(8 * (NRB - 1) + 2, H)]
    with TileContext(nc) as tc:
        with tc.tile_pool(name="wp", bufs=1) as wp, \
             tc.tile_pool(name="xpp", bufs=2) as xpp, \
             tc.tile_pool(name="pp", bufs=6, space="PSUM") as pp, \
             tc.tile_pool(name="wup", bufs=1, space="PSUM") as wup, \
             tc.tile_pool(name="op", bufs=4) as outp:
            # PE warm-up: dummy matmuls with no data deps so the HAM clock
            # gate is at 8/8 by the time real matmuls start (and stays there
            # until the first image's data lands).
            wu_in = wp.tile([C, 64], f16)
            nc.vector.memset(wu_in, 0.0)
            wu_ps = wup.tile([32, 64], f32)
            for _ in range(100):
                nc.tensor.matmul(wu_ps, wu_in[:, :32], wu_in[:, :64],
                                 start=True, stop=True)
            # img0 chunk0 via fast HWDGE path (sync can't cast: stage fp32,
            # cast on DVE); remaining chunks via gpsimd casting DMA
            wt = wp.tile([C, 9 * O], f16)
            xs0 = wp.tile([C, 10, W], f32)
            nc.sync.dma_start(out=xs0, in_=x[0, :, 0:10, :])
            nc.scalar.dma_start(out=wt, in_=w[:, :])
            for img in range(PB):
                xp = xpp.tile([C, HP, WP], f16)
                nc.vector.memset(xp[:, 0, :], 0.0)
                nc.vector.memset(xp[:, HP - 1, :], 0.0)
                nc.vector.memset(xp[:, 1:HP - 1, 0], 0.0)
                nc.vector.memset(xp[:, 1:HP - 1, WP - 1], 0.0)
                # fp32 -> fp16 casting DMA straight into the padded tile
                if img == 0:
                    for ci, (r0, r1) in enumerate(CH0):
                        if ci == 0:
                            nc.vector.tensor_copy(
                                out=xp[:, 1:11, 1:W + 1], in_=xs0)
                            continue
                        nc.gpsimd.dma_start(
                            out=xp[:, r0 + 1:r1 + 1, 1:W + 1],
                            in_=x[img, :, r0:r1, :])
                else:
                    nc.gpsimd.dma_start(out=xp[:, 1:H + 1, 1:W + 1],
                                        in_=x[img, :, :, :])
                for rb in range(NRB):
                    for oh in range(O // 128):
                        ps = pp.tile([128, RB, W], f32)
                        for ki in range(K):
                            for kj in range(K):
                                kidx = ki * K + kj
                                rhs = xp[:, rb * RB + ki: rb * RB + ki + RB,
                                         kj: kj + W]
                                lhsT = wt[:, kidx * O + oh * 128:
                                          kidx * O + oh * 128 + 128]
                                nc.tensor.matmul(ps, lhsT, rhs,
                                                 start=(kidx == 0),
                                                 stop=(kidx == K * K - 1))
                        yt = outp.tile([128, RB, W], f32)
                        if img == PB - 1 and rb == NRB - 1:
                            # tail latency: drain the last PSUM tiles with
                            # both engines in parallel
                            nc.vector.tensor_copy(out=yt[:, :RB // 2, :],
                                                  in_=ps[:, :RB // 2, :])
                            nc.scalar.copy(out=yt[:, RB // 2:, :],
                                           in_=ps[:, RB // 2:, :])
                        elif oh % 2 == 0:
                            nc.vector.tensor_copy(out=yt, in_=ps)
                        else:
                            nc.scalar.copy(out=yt, in_=ps)
                        nc.sync.dma_start(
                            out=y[img, oh * 128:(oh + 1) * 128,
                                  rb * RB:(rb + 1) * RB, :],
                            in_=yt)
    nc.compile()
    return nc


def _prep_weights(weight, phases, disks):
    """dorefa weight quantize + fold phases/disks into the conv kernel."""
    t = np.tanh(weight.astype(np.float32))
    t = t / (2.0 * np.max(np.abs(t))) + 0.5
    wq = (np.round(t * QN) / np.float32(QN)).astype(np.float32)
    s = np.sin(phases.astype(np.float32))[0, 0]        # (C,K,K)
    d0 = disks[0, 0, ..., 0].astype(np.float32)
    d1 = disks[0, 0, ..., 1].astype(np.float32)
    k_mul = wq * (s * (d0 + d1) * 0.5)[None]           # (O,C,K,K)
    # lhsT layout: [c, kidx*O + o]
    wsb = np.ascontiguousarray(
        k_mul.transpose(1, 2, 3, 0).reshape(C, 9 * O)).astype(np.float16)
    coef = (d0 - d1) * 0.25                            # (C,K,K)
    return wsb, wq, coef


def _square_terms(x, wq, coef):
    """Generic-disk correction (zero for ideal disks): conv(x_q^2, coef)
    broadcast over O, plus per-O constant sum(w_q^2 * coef)."""
    xq = np.round(np.clip(x, 0.0, 1.0) * QN) / np.float32(QN)
    x2 = (xq * xq).astype(np.float32)
    bsz = x.shape[0]
    x2p = np.zeros((bsz, C, H + 2, W + 2), np.float32)
    x2p[:, :, 1:H + 1, 1:W + 1] = x2
    y_sq = np.zeros((bsz, H, W), np.float32)
    for ki in range(K):
        for kj in range(K):
            y_sq += np.einsum("bchw,c->bhw",
                              x2p[:, :, ki:ki + H, kj:kj + W],
                              coef[:, ki, kj], optimize=True)
    w_term = np.einsum("ockk,ckk->o", wq * wq, coef)
    return y_sq[:, None] + w_term[None, :, None, None]


def kernel(x, weight, phases, disks):
    x = np.asarray(x)
    wsb, wq, coef = _prep_weights(np.asarray(weight), np.asarray(phases),
                                  np.asarray(disks))
    if "nc" not in _CACHE:
        _CACHE["nc"] = _build_nc()
    nc = _CACHE["nc"]
    in_maps = [{"x": np.ascontiguousarray(x[c * PB:(c + 1) * PB]), "w": wsb}
               for c in range(N_CORES)]
    res = run_bass_kernel_spmd(nc, in_maps, list(range(N_CORES)))
    y = np.concatenate([res.results[c]["y"] for c in range(N_CORES)], axis=0)
    if np.any(coef != 0.0):
        y = y + _square_terms(x, wq, coef)
    return y.astype(np.float32)


# revision 19
# speedup vs baseline: 1.0178x; 1.0023x over previous
"""OAdder2d_Q (oconv, 16-bit dorefa quant) as an 8-core Trainium2 Bass kernel.

Math: with ideal disks the op is a 3x3/pad1 conv with effective kernel
w_q * sin(phases)*(d0+d1)/2.  The tiny weight transform (tanh/dorefa +
phase fold) runs on host; the conv runs on device as 9 shifted matmuls
(one per kernel tap) accumulating in PSUM, operands in fp16.

The 16-bit input quantize round(clip(x)*65535)/65535 perturbs x by at most
7.6e-6 relative -- far below fp16's 2.4e-4 ulp -- so casting x straight to
fp16 is numerically indistinguishable from quantize-then-cast (verified:
6.7e-4 vs 6.4e-4 scale-relative error).  The input path is therefore a
single dtype-casting DMA into a zero-padded fp16 SBUF tile.

Sharding: data-parallel over batch, 32 images -> 4 per core, weights
replicated.
"""

import sys

if "/opt/trn_rl_repo" not in sys.path:
    sys.path.insert(0, "/opt/trn_rl_repo")

import numpy as np

import concourse.bacc as bacc
import concourse.mybir as mybir
from concourse.tile import TileContext
from concourse.bass_utils import run_bass_kernel_spmd

N_CORES = 8
B, C, O, K, H, W = 32, 128, 256, 3, 56, 56
PB = B // N_CORES              # images per core
HP, WP = H + 2, W + 2          # padded spatial
RB = 8                         # output rows per psum tile
NRB = H // RB                  # row blocks per image
QN = 65535.0                   # 2^16 - 1

f32 = mybir.dt.float32
f16 = mybir.dt.float16

_CACHE = {}


def _build_nc():
    nc = bacc.Bacc("TRN2", target_bir_lowering=False, debug=False,
                   num_devices=N_CORES)
    x = nc.dram_tensor("x", (PB, C, H, W), f32, kind="ExternalInput")
    w = nc.dram_tensor("w", (C, 9 * O), f16, kind="ExternalInput")
    y = nc.dram_tensor("y", (PB, O, H, W), f32, kind="ExternalOutput")

    # img0 input row chunks, halo-aligned so chunk k unlocks row-block k:
    # rb k's matmuls read padded rows [8k, 8k+10] = x rows [8k-1, 8k+9]
    CH0 = [(0, 10)] + [(8 * k + 2, 8 * k + 10) for k in range(1, NRB - 1)] \
        + [(8 * (NRB - 1) + 2, H)]
    with TileContext(nc) as tc:
        with tc.tile_pool(name="wp", bufs=1) as wp, \
             tc.tile_pool(name="xpp", bufs=2) as xpp, \
             tc.tile_pool(name="pp", bufs=7, space="PSUM") as pp, \
             tc.tile_pool(name="wup", bufs=1, space="PSUM") as wup, \
             tc.tile_pool(name="op", bufs=4) as outp:
            # PE warm-up: dummy matmuls with no data deps so the HAM clock
            # gate is at 8/8 by the time real matmuls start (and stays there
            # until the first image's data lands).
            wu_in = wp.tile([C, 64], f16)
            nc.vector.memset(wu_in, 0.0)
            wu_ps = wup.tile([32, 64], f32)
            for _ in range(100):
                nc.tensor.matmul(wu_ps, wu_in[:, :32], wu_in[:, :64],
                                 start=True, stop=True)
            # img0 chunk0 via fast HWDGE path (sync can't cast: stage fp32,
            # cast on DVE); remaining chunks via gpsimd casting DMA
            wt = wp.tile([C, 9 * O], f16)
            xs0 = wp.tile([C, 10, W], f32)
            nc.sync.dma_start(out=xs0, in_=x[0, :, 0:10, :])
            nc.scalar.dma_start(out=wt, in_=w[:, :])
            for img in range(PB):
                xp = xpp.tile([C, HP, WP], f16)
                nc.vector.memset(xp[:, 0, :], 0.0)
                nc.vector.memset(xp[:, HP - 1, :], 0.0)
                nc.vector.memset(xp[:, 1:HP - 1, 0], 0.0)
                nc.vector.memset(xp[:, 1:HP - 1, WP - 1], 0.0)
                # fp32 -> fp16 casting DMA straight into the padded tile
                if img == 0:
                    for ci, (r0, r1) in enumerate(CH0):
                        if ci == 0:
                            nc.vector.tensor_copy(
                                out=xp[:, 1:11, 1:W + 1], in_=xs0)
                            continue
                        nc.gpsimd.dma_start(
                            out=xp[:, r0 + 1:r1 + 1, 1:W + 1],
                            in_=x[img, :, r0:r1, :])
                else:
                    nc.gpsimd.dma_start(out=xp[:, 1:H + 1, 1:W + 1],
                                        in_=x[img, :, :, :])
                for rb in range(NRB):
                    for oh in range(O // 128):
                        ps = pp.tile([128, RB, W], f32)
                        for ki in range(K):
                            for kj in range(K):
                                kidx = ki * K + kj
                                rhs = xp[:, rb * RB + ki: rb * RB + ki + RB,
                                         kj: kj + W]
                                lhsT = wt[:, kidx * O + oh * 128:
                                          kidx * O + oh * 128 + 128]
                                nc.tensor.matmul(ps, lhsT, rhs,
                                                 start=(kidx == 0),
                                                 stop=(kidx == K * K - 1))
                        yt = outp.tile([128, RB, W], f32)
                        if img == PB - 1 and rb == NRB - 1:
                            # tail latency: drain the last PSUM tiles with
                            # both engines in parallel
                            nc.vector.tensor_copy(out=yt[:, :RB // 2, :],
                                                  in_=ps[:, :RB // 2, :])
                            nc.scalar.copy(out=yt[:, RB // 2:, :],
                                           in_=ps[:, RB // 2:, :])
                        elif oh % 2 == 0:
                            nc.vector.tensor_copy(out=yt, in_=ps)
                        else:
                            nc.scalar.copy(out=yt, in_=ps)
                        nc.sync.dma_start(
                            out=y[img, oh * 128:(oh + 1) * 128,
                                  rb * RB:(rb + 1) * RB, :],
                            in_=yt)
    nc.compile()
    return nc


def _prep_weights(weight, phases, disks):
    """dorefa weight quantize + fold phases/disks into the conv kernel."""
    t = np.tanh(weight.astype(np.float32))
    t = t / (2.0 * np.max(np.abs(t))) + 0.5
    wq = (np.round(t * QN) / np.float32(QN)).astype(np.float32)
    s = np.sin(phases.astype(np.float32))[0, 0]        # (C,K,K)
    d0 = disks[0, 0, ..., 0].astype(np.float32)
    d1 = disks[0, 0, ..., 1].astype(np.float32)
    k_mul = wq * (s * (d0 + d1) * 0.5)[None]           # (O,C,K,K)
    # lhsT layout: [c, kidx*O + o]
    wsb = np.ascontiguousarray(
        k_mul.transpose(1, 2, 3, 0).reshape(C, 9 * O)).astype(np.float16)
    coef = (d0 - d1) * 0.25                            # (C,K,K)
    return wsb, wq, coef


def _square_terms(x, wq, coef):
    """Generic-disk correction (zero for ideal disks): conv(x_q^2, coef)
    broadcast over O, plus per-O constant sum(w_q^2 * coef)."""
    xq = np.round(np.clip(x, 0.0, 1.0) * QN) / np.float32(QN)
    x2 = (xq * xq).astype(np.float32)
    bsz = x.shape[0]
    x2p = np.zeros((bsz, C, H + 2, W + 2), np.float32)
    x2p[:, :, 1:H + 1, 1:W + 1] = x2
    y_sq = np.zeros((bsz, H, W), np.float32)
    for ki in range(K):
        for kj in range(K):
            y_sq += np.einsum("bchw,c->bhw",
                              x2p[:, :, ki:ki + H, kj:kj + W],
                              coef[:, ki, kj], optimize=True)
    w_term = np.einsum("ockk,ckk->o", wq * wq, coef)
    return y_sq[:, None] + w_term[None, :, None, None]


def kernel(x, weight, phases, disks):
    x = np.asarray(x)
    wsb, wq, coef = _prep_weights(np.asarray(weight), np.asarray(phases),
                                  np.asarray(disks))
    if "nc" not in _CACHE:
        _CACHE["nc"] = _build_nc()
    nc = _CACHE["nc"]
    in_maps = [{"x": np.ascontiguousarray(x[c * PB:(c + 1) * PB]), "w": wsb}
               for c in range(N_CORES)]
    res = run_bass_kernel_spmd(nc, in_maps, list(range(N_CORES)))
    y = np.concatenate([res.results[c]["y"] for c in range(N_CORES)], axis=0)
    if np.any(coef != 0.0):
        y = y + _square_terms(x, wq, coef)
    return y.astype(np.float32)


# revision 21
# speedup vs baseline: 1.0184x; 1.0006x over previous
"""OAdder2d_Q (oconv, 16-bit dorefa quant) as an 8-core Trainium2 Bass kernel.

Math: with ideal disks the op is a 3x3/pad1 conv with effective kernel
w_q * sin(phases)*(d0+d1)/2.  The tiny weight transform (tanh/dorefa +
phase fold) runs on host; the conv runs on device as 9 shifted matmuls
(one per kernel tap) accumulating in PSUM, operands in fp16.

The 16-bit input quantize round(clip(x)*65535)/65535 perturbs x by at most
7.6e-6 relative -- far below fp16's 2.4e-4 ulp -- so casting x straight to
fp16 is numerically indistinguishable from quantize-then-cast (verified:
6.7e-4 vs 6.4e-4 scale-relative error).  The input path is therefore a
single dtype-casting DMA into a zero-padded fp16 SBUF tile.

Sharding: data-parallel over batch, 32 images -> 4 per core, weights
replicated.
"""

import sys

if "/opt/trn_rl_repo" not in sys.path:
    sys.path.insert(0, "/opt/trn_rl_repo")

import numpy as np

import concourse.bacc as bacc
import concourse.mybir as mybir
from concourse.tile import TileContext
from concourse.vector_clock import ScopedClock
from concourse.bass_utils import run_bass_kernel_spmd


class _FastExitTileContext(TileContext):
    """TileContext whose exit drains + barriers but skips the end-of-kernel
    semaphore clear and trailing all-engine barrier (~5us).  Safe here: the
    kernel executes once per NEFF load (run_bass_via_pjrt builds a fresh
    executable per call), and semaphores are zeroed at load."""

    def _drain_and_barrier(self, tick_clock, wait_clock):
        drain_inst = self.nc.sync.drain()
        wait_clock.add_sem_waits(
            drain_inst.ins, ScopedClock({None: tick_clock.global_clock}))
        self.nc.all_engine_barrier()
        assert self.sems is not None
        popped = self.nc._tile_sem_poison_stack.pop()
        assert popped is self._sem_poison

N_CORES = 8
B, C, O, K, H, W = 32, 128, 256, 3, 56, 56
PB = B // N_CORES              # images per core
HP, WP = H + 2, W + 2          # padded spatial
RB = 8                         # output rows per psum tile
NRB = H // RB                  # row blocks per image
QN = 65535.0                   # 2^16 - 1

f32 = mybir.dt.float32
f16 = mybir.dt.float16

_CACHE = {}


def _build_nc():
    nc = bacc.Bacc("TRN2", target_bir_lowering=False, debug=False,
                   num_devices=N_CORES)
    x = nc.dram_tensor("x", (PB, C, H, W), f32, kind="ExternalInput")
    w = nc.dram_tensor("w", (C, 9 * O), f16, kind="ExternalInput")
    y = nc.dram_tensor("y", (PB, O, H, W), f32, kind="ExternalOutput")

    # img0 input row chunks, halo-aligned so chunk k unlocks row-block k:
    # rb k's matmuls read padded rows [8k, 8k+10] = x rows [8k-1, 8k+9]
    CH0 = [(0, 10)] + [(8 * k + 2, 8 * k + 10) for k in range(1, NRB - 1)] \
        + [(8 * (NRB - 1) + 2, H)]
    with _FastExitTileContext(nc) as tc:
        with tc.tile_pool(name="wp", bufs=1) as wp, \
             tc.tile_pool(name="xpp", bufs=2) as xpp, \
             tc.tile_pool(name="pp", bufs=7, space="PSUM") as pp, \
             tc.tile_pool(name="wup", bufs=1, space="PSUM") as wup, \
             tc.tile_pool(name="op", bufs=4) as outp:
            # PE warm-up: dummy matmuls with no data deps so the HAM clock
            # gate is at 8/8 by the time real matmuls start (and stays there
            # until the first image's data lands).
            wu_in = wp.tile([C, 64], f16)
            nc.vector.memset(wu_in, 0.0)
            wu_ps = wup.tile([32, 64], f32)
            for _ in range(100):
                nc.tensor.matmul(wu_ps, wu_in[:, :32], wu_in[:, :64],
                                 start=True, stop=True)
            # img0 chunk0 via fast HWDGE path (sync can't cast: stage fp32,
            # cast on DVE); remaining chunks via gpsimd casting DMA
            wt = wp.tile([C, 9 * O], f16)
            xs0 = wp.tile([C, 10, W], f32)
            nc.sync.dma_start(out=xs0, in_=x[0, :, 0:10, :])
            nc.scalar.dma_start(out=wt, in_=w[:, :])
            for img in range(PB):
                xp = xpp.tile([C, HP, WP], f16)
                nc.vector.memset(xp[:, 0, :], 0.0)
                nc.vector.memset(xp[:, HP - 1, :], 0.0)
                nc.vector.memset(xp[:, 1:HP - 1, 0], 0.0)
                nc.vector.memset(xp[:, 1:HP - 1, WP - 1], 0.0)
                # fp32 -> fp16 casting DMA straight into the padded tile
                if img == 0:
                    for ci, (r0, r1) in enumerate(CH0):
                        if ci == 0:
                            nc.vector.tensor_copy(
                                out=xp[:, 1:11, 1:W + 1], in_=xs0)
                            continue
                        nc.gpsimd.dma_start(
                            out=xp[:, r0 + 1:r1 + 1, 1:W + 1],
                            in_=x[img, :, r0:r1, :])
                else:
                    nc.gpsimd.dma_start(out=xp[:, 1:H + 1, 1:W + 1],
                                        in_=x[img, :, :, :])
                for rb in range(NRB):
                    for oh in range(O // 128):
                        ps = pp.tile([128, RB, W], f32)
                        for ki in range(K):
                            for kj in range(K):
                                kidx = ki * K + kj
                                rhs = xp[:, rb * RB + ki: rb * RB + ki + RB,
                                         kj: kj + W]
                                lhsT = wt[:, kidx * O + oh * 128:
                                          kidx * O + oh * 128 + 128]
                                nc.tensor.matmul(ps, lhsT, rhs,
                                                 start=(kidx == 0),
                                                 stop=(kidx == K * K - 1))
                        yt = outp.tile([128, RB, W], f32)
                        if img == PB - 1 and rb == NRB - 1:
                            # tail latency: drain the last PSUM tiles with
                            # both engines in parallel
                            nc.vector.tensor_copy(out=yt[:, :RB // 2, :],
                                                  in_=ps[:, :RB // 2, :])
                            nc.scalar.copy(out=yt[:, RB // 2:, :],
                                           in_=ps[:, RB // 2:, :])
                        elif oh % 2 == 0:
                            nc.vector.tensor_copy(out=yt, in_=ps)
                        else:
                            nc.scalar.copy(out=yt, in_=ps)
                        nc.sync.dma_start(
                            out=y[img, oh * 128:(oh + 1) * 128,
                                  rb * RB:(rb + 1) * RB, :],
                            in_=yt)
    nc.compile()
    return nc


def _prep_weights(weight, phases, disks):
    """dorefa weight quantize + fold phases/disks into the conv kernel."""
    t = np.tanh(weight.astype(np.float32))
    t = t / (2.0 * np.max(np.abs(t))) + 0.5
    wq = (np.round(t * QN) / np.float32(QN)).astype(np.float32)
    s = np.sin(phases.astype(np.float32))[0, 0]        # (C,K,K)
    d0 = disks[0, 0, ..., 0].astype(np.float32)
    d1 = disks[0, 0, ..., 1].astype(np.float32)
    k_mul = wq * (s * (d0 + d1) * 0.5)[None]           # (O,C,K,K)
    # lhsT layout: [c, kidx*O + o]
    wsb = np.ascontiguousarray(
        k_mul.transpose(1, 2, 3, 0).reshape(C, 9 * O)).astype(np.float16)
    coef = (d0 - d1) * 0.25                            # (C,K,K)
    return wsb, wq, coef


def _square_terms(x, wq, coef):
    """Generic-disk correction (zero for ideal disks): conv(x_q^2, coef)
    broadcast over O, plus per-O constant sum(w_q^2 * coef)."""
    xq = np.round(np.clip(x, 0.0, 1.0) * QN) / np.float32(QN)
    x2 = (xq * xq).astype(np.float32)
    bsz = x.shape[0]
    x2p = np.zeros((bsz, C, H + 2, W + 2), np.float32)
    x2p[:, :, 1:H + 1, 1:W + 1] = x2
    y_sq = np.zeros((bsz, H, W), np.float32)
    for ki in range(K):
        for kj in range(K):
            y_sq += np.einsum("bchw,c->bhw",
                              x2p[:, :, ki:ki + H, kj:kj + W],
                              coef[:, ki, kj], optimize=True)
    w_term = np.einsum("ockk,ckk->o", wq * wq, coef)
    return y_sq[:, None] + w_term[None, :, None, None]


def kernel(x, weight, phases, disks):
    x = np.asarray(x)
    wsb, wq, coef = _prep_weights(np.asarray(weight), np.asarray(phases),
                                  np.asarray(disks))
    if "nc" not in _CACHE:
        _CACHE["nc"] = _build_nc()
    nc = _CACHE["nc"]
    in_maps = [{"x": np.ascontiguousarray(x[c * PB:(c + 1) * PB]), "w": wsb}
               for c in range(N_CORES)]
    res = run_bass_kernel_spmd(nc, in_maps, list(range(N_CORES)))
    y = np.concatenate([res.results[c]["y"] for c in range(N_CORES)], axis=0)
    if np.any(coef != 0.0):
        y = y + _square_terms(x, wq, coef)
    return y.astype(np.float32)


# revision 22
# speedup vs baseline: 1.0184x; 1.0001x over previous
"""OAdder2d_Q (oconv, 16-bit dorefa quant) as an 8-core Trainium2 Bass kernel.

Math: with ideal disks the op is a 3x3/pad1 conv with effective kernel
w_q * sin(phases)*(d0+d1)/2.  The tiny weight transform (tanh/dorefa +
phase fold) runs on host; the conv runs on device as 9 shifted matmuls
(one per kernel tap) accumulating in PSUM, operands in fp16.

The 16-bit input quantize round(clip(x)*65535)/65535 perturbs x by at most
7.6e-6 relative -- far below fp16's 2.4e-4 ulp -- so casting x straight to
fp16 is numerically indistinguishable from quantize-then-cast (verified:
6.7e-4 vs 6.4e-4 scale-relative error).  The input path is therefore a
single dtype-casting DMA into a zero-padded fp16 SBUF tile.

Sharding: data-parallel over batch, 32 images -> 4 per core, weights
replicated.
"""

import sys

if "/opt/trn_rl_repo" not in sys.path:
    sys.path.insert(0, "/opt/trn_rl_repo")

import numpy as np

import concourse.bacc as bacc
import concourse.mybir as mybir
from concourse.tile import TileContext
from concourse.vector_clock import ScopedClock
from concourse.bass_utils import run_bass_kernel_spmd


class _FastExitTileContext(TileContext):
    """TileContext whose exit drains + barriers but skips the end-of-kernel
    semaphore clear and trailing all-engine barrier (~5us).  Safe here: the
    kernel executes once per NEFF load (run_bass_via_pjrt builds a fresh
    executable per call), and semaphores are zeroed at load."""

    def _drain_and_barrier(self, tick_clock, wait_clock):
        drain_inst = self.nc.sync.drain()
        wait_clock.add_sem_waits(
            drain_inst.ins, ScopedClock({None: tick_clock.global_clock}))
        self.nc.all_engine_barrier(sem_only=True)
        assert self.sems is not None
        popped = self.nc._tile_sem_poison_stack.pop()
        assert popped is self._sem_poison

N_CORES = 8
B, C, O, K, H, W = 32, 128, 256, 3, 56, 56
PB = B // N_CORES              # images per core
HP, WP = H + 2, W + 2          # padded spatial
RB = 8                         # output rows per psum tile
NRB = H // RB                  # row blocks per image
QN = 65535.0                   # 2^16 - 1

f32 = mybir.dt.float32
f16 = mybir.dt.float16

_CACHE = {}


def _build_nc():
    nc = bacc.Bacc("TRN2", target_bir_lowering=False, debug=False,
                   num_devices=N_CORES)
    x = nc.dram_tensor("x", (PB, C, H, W), f32, kind="ExternalInput")
    w = nc.dram_tensor("w", (C, 9 * O), f16, kind="ExternalInput")
    y = nc.dram_tensor("y", (PB, O, H, W), f32, kind="ExternalOutput")

    # img0 input row chunks, halo-aligned so chunk k unlocks row-block k:
    # rb k's matmuls read padded rows [8k, 8k+10] = x rows [8k-1, 8k+9]
    CH0 = [(0, 10)] + [(8 * k + 2, 8 * k + 10) for k in range(1, NRB - 1)] \
        + [(8 * (NRB - 1) + 2, H)]
    with _FastExitTileContext(nc) as tc:
        with tc.tile_pool(name="wp", bufs=1) as wp, \
             tc.tile_pool(name="xpp", bufs=2) as xpp, \
             tc.tile_pool(name="pp", bufs=7, space="PSUM") as pp, \
             tc.tile_pool(name="wup", bufs=1, space="PSUM") as wup, \
             tc.tile_pool(name="op", bufs=4) as outp:
            # PE warm-up: dummy matmuls with no data deps so the HAM clock
            # gate is at 8/8 by the time real matmuls start (and stays there
            # until the first image's data lands).
            wu_in = wp.tile([C, 64], f16)
            nc.vector.memset(wu_in, 0.0)
            wu_ps = wup.tile([32, 64], f32)
            for _ in range(100):
                nc.tensor.matmul(wu_ps, wu_in[:, :32], wu_in[:, :64],
                                 start=True, stop=True)
            # img0 chunk0 via fast HWDGE path (sync can't cast: stage fp32,
            # cast on DVE); remaining chunks via gpsimd casting DMA
            wt = wp.tile([C, 9 * O], f16)
            xs0 = wp.tile([C, 10, W], f32)
            nc.sync.dma_start(out=xs0, in_=x[0, :, 0:10, :])
            nc.scalar.dma_start(out=wt, in_=w[:, :])
            for img in range(PB):
                xp = xpp.tile([C, HP, WP], f16)
                nc.vector.memset(xp[:, 0, :], 0.0)
                nc.vector.memset(xp[:, HP - 1, :], 0.0)
                nc.vector.memset(xp[:, 1:HP - 1, 0], 0.0)
                nc.vector.memset(xp[:, 1:HP - 1, WP - 1], 0.0)
                # fp32 -> fp16 casting DMA straight into the padded tile
                if img == 0:
                    for ci, (r0, r1) in enumerate(CH0):
                        if ci == 0:
                            nc.vector.tensor_copy(
                                out=xp[:, 1:11, 1:W + 1], in_=xs0)
                            continue
                        nc.gpsimd.dma_start(
                            out=xp[:, r0 + 1:r1 + 1, 1:W + 1],
                            in_=x[img, :, r0:r1, :])
                else:
                    nc.gpsimd.dma_start(out=xp[:, 1:H + 1, 1:W + 1],
                                        in_=x[img, :, :, :])
                for rb in range(NRB):
                    for oh in range(O // 128):
                        ps = pp.tile([128, RB, W], f32)
                        for ki in range(K):
                            for kj in range(K):
                                kidx = ki * K + kj
                                rhs = xp[:, rb * RB + ki: rb * RB + ki + RB,
                                         kj: kj + W]
                                lhsT = wt[:, kidx * O + oh * 128:
                                          kidx * O + oh * 128 + 128]
                                nc.tensor.matmul(ps, lhsT, rhs,
                                                 start=(kidx == 0),
                                                 stop=(kidx == K * K - 1))
                        yt = outp.tile([128, RB, W], f32)
                        if img == PB - 1 and rb == NRB - 1:
                            # tail latency: drain the last PSUM tiles with
                            # both engines in parallel
                            nc.vector.tensor_copy(out=yt[:, :RB // 2, :],
                                                  in_=ps[:, :RB // 2, :])
                            nc.scalar.copy(out=yt[:, RB // 2:, :],
                                           in_=ps[:, RB // 2:, :])
                        elif oh % 2 == 0:
                            nc.vector.tensor_copy(out=yt, in_=ps)
                        else:
                            nc.scalar.copy(out=yt, in_=ps)
                        nc.sync.dma_start(
                            out=y[img, oh * 128:(oh + 1) * 128,
                                  rb * RB:(rb + 1) * RB, :],
                            in_=yt)
    nc.compile()
    return nc


def _prep_weights(weight, phases, disks):
    """dorefa weight quantize + fold phases/disks into the conv kernel."""
    t = np.tanh(weight.astype(np.float32))
    t = t / (2.0 * np.max(np.abs(t))) + 0.5
    wq = (np.round(t * QN) / np.float32(QN)).astype(np.float32)
    s = np.sin(phases.astype(np.float32))[0, 0]        # (C,K,K)
    d0 = disks[0, 0, ..., 0].astype(np.float32)
    d1 = disks[0, 0, ..., 1].astype(np.float32)
    k_mul = wq * (s * (d0 + d1) * 0.5)[None]           # (O,C,K,K)
    # lhsT layout: [c, kidx*O + o]
    wsb = np.ascontiguousarray(
        k_mul.transpose(1, 2, 3, 0).reshape(C, 9 * O)).astype(np.float16)
    coef = (d0 - d1) * 0.25                            # (C,K,K)
    return wsb, wq, coef


def _square_terms(x, wq, coef):
    """Generic-disk correction (zero for ideal disks): conv(x_q^2, coef)
    broadcast over O, plus per-O constant sum(w_q^2 * coef)."""
    xq = np.round(np.clip(x, 0.0, 1.0) * QN) / np.float32(QN)
    x2 = (xq * xq).astype(np.float32)
    bsz = x.shape[0]
    x2p = np.zeros((bsz, C, H + 2, W + 2), np.float32)
    x2p[:, :, 1:H + 1, 1:W + 1] = x2
    y_sq = np.zeros((bsz, H, W), np.float32)
    for ki in range(K):
        for kj in range(K):
            y_sq += np.einsum("bchw,c->bhw",
                              x2p[:, :, ki:ki + H, kj:kj + W],
                              coef[:, ki, kj], optimize=True)
    w_term = np.einsum("ockk,ckk->o", wq * wq, coef)
    return y_sq[:, None] + w_term[None, :, None, None]


def kernel(x, weight, phases, disks):
    x = np.asarray(x)
    wsb, wq, coef = _prep_weights(np.asarray(weight), np.asarray(phases),
                                  np.asarray(disks))
    if "nc" not in _CACHE:
        _CACHE["nc"] = _build_nc()
    nc = _CACHE["nc"]
    in_maps = [{"x": np.ascontiguousarray(x[c * PB:(c + 1) * PB]), "w": wsb}
               for c in range(N_CORES)]
    res = run_bass_kernel_spmd(nc, in_maps, list(range(N_CORES)))
    y = np.concatenate([res.results[c]["y"] for c in range(N_CORES)], axis=0)
    if np.any(coef != 0.0):
        y = y + _square_terms(x, wq, coef)
    return y.astype(np.float32)
